# revision 21
# baseline (speedup 1.0000x reference)
"""AdderNet BasicBlock kernel for Trainium2, co-sharded across 8 cores.

Per core (co-shard CO=8 of 64 output channels):
  conv[co,n,p] = -sum_{ci,kh,kw} |x[n,ci,p+k-1] - w[co,ci,kh,kw]|   (pad=1)
  BN train-mode over (n,h,w) per co, then ReLU.

Formulation (v2):
  Taps are split between engines:
    ACT taps (2 of 9): |d| = Abs(x + (-w)) directly  -> PE weight -1
    DVE taps (7 of 9): relu(x-w) = tensor_scalar(sub, max 0), bf16 4x mode
        |d| = 2*relu(d) - d            -> PE weight -2, Box correction +1
  conv = -(sum_ACT |d| + 2*sum_DVE relu(d)) + Box_D - SwD
  SwD is a per-channel constant -> dropped (BN is shift-invariant per channel).

PE: 4-way column tiling. Position j holds co j (tile A) / co 4+j (tile B);
ones-reduce over 128 partitions (2 image groups x 64 ci), 512-col blocks,
PSUM accumulates 9 taps. Box (7 DVE taps over x) streams into tile A after
its evacuation, at a per-half rotating position.

BN stage 2 as before: bounce conv to DRAM, reload as [(co,n), hw],
replicated-selector matmul stats, fused affine+relu on ACT.
"""
from contextlib import ExitStack

import numpy as np

import concourse.bass as bass
import concourse.tile as tile
import concourse.mybir as mybir

F32 = mybir.dt.float32
BF16 = mybir.dt.bfloat16
BN_EPS = 1e-5

N, CI, H, W = 16, 64, 32, 32
CO = 8          # output channels per core
HW = H * W      # 1024
PADH, PADW = H + 2, W + 4   # 34 x 36 (2 extra zero cols: alignment + even dims)

N_HALVES = 4
JPH = 2                      # images per group per half
TCOLS = JPH * HW             # 2048 free-dim per stream
NB = TCOLS // 512            # 512-col matmul blocks per stream

ACT_TAPS = (1, 7)            # (0,1),(2,1): kw=1 taps go to ACT (Abs direct)
# interleave ACT taps for engine overlap; DVE taps use xb0 (kw even) or xb1
TAP_ORDER = (0, 2, 3, 1, 4, 5, 7, 6, 8)


def split_multiwaits(nc, max_waits=1):
    """This container's walrus rejects >1 semaphore wait per instruction.
    Hoist extras into standalone NoOps on the same (in-order) engine."""
    n_split = 0
    for f in nc.m.functions:
        for b in f.blocks:
            insts = list(b.instructions)
            changed = False
            new = []
            for inst in insts:
                si = inst.sync_info
                waits = list(si.on_wait) if si and si.on_wait else []
                if len(waits) > max_waits:
                    changed = True
                    n_split += 1
                    for w in waits[: len(waits) - max_waits]:
                        new.append(mybir.InstNoOp(
                            name=nc.get_next_instruction_name(),
                            engine=inst.engine, ins=[], outs=[],
                            sync_info=mybir.SyncInfo(on_wait=[w], on_update=[]),
                        ))
                    inst.sync_info = mybir.SyncInfo(
                        on_wait=waits[len(waits) - max_waits:],
                        on_update=list(si.on_update) if si.on_update else [],
                    )
                new.append(inst)
            if changed:
                b.instructions = new
    return n_split


def build_nc(t_bufs=16, debug=False):
    """One core's SPMD program."""
    nc = bass.Bass()
    x = nc.declare_dram_parameter("x", [N, CI, H, W], F32, isOutput=False)
    w = nc.declare_dram_parameter("w", [CO, CI, 3, 3], F32, isOutput=False)
    gamma = nc.declare_dram_parameter("gamma", [CO], F32, isOutput=False)
    beta = nc.declare_dram_parameter("beta", [CO], F32, isOutput=False)
    selcor_in = nc.declare_dram_parameter("selcor", [128, 128], F32,
                                          isOutput=False)
    out = nc.declare_dram_parameter("out", [N, CO, H, W], F32, isOutput=True)
    if debug:
        dbg_cs = nc.declare_dram_parameter("dbg_cs", [128, HW], F32,
                                           isOutput=True)
        dbg_box = nc.declare_dram_parameter("dbg_box", [128, HW], F32,
                                            isOutput=True)

    with tile.TileContext(nc) as tc, ExitStack() as ctx:
        singles = ctx.enter_context(tc.tile_pool(name="singles", bufs=1))
        xspool = ctx.enter_context(tc.tile_pool(name="xstage", bufs=3))
        xbpool = ctx.enter_context(tc.tile_pool(name="xb", bufs=3))
        tpool = ctx.enter_context(tc.tile_pool(name="tpool", bufs=t_bufs))
        scpool = ctx.enter_context(tc.tile_pool(name="scr", bufs=2))
        sbpool = ctx.enter_context(tc.tile_pool(name="scrbox", bufs=2))
        pspool = ctx.enter_context(tc.tile_pool(name="psum", bufs=2,
                                                space="PSUM"))
        spool = ctx.enter_context(tc.tile_pool(name="stage2", bufs=1))
        dpool = ctx.enter_context(tc.tile_pool(name="dram", bufs=1,
                                               space="DRAM"))

        # ---- constants ----
        def ones_pair(val, nm):
            t = singles.tile([128, 2], BF16, name=nm)
            nc.vector.memset(t[:, :], 0.0)
            nc.vector.memset(t[0:64, 0:1], val)
            nc.vector.memset(t[64:128, 1:2], val)
            return t
        sel_m2 = ones_pair(-2.0, "sel_m2")   # DVE relu streams
        sel_m1 = ones_pair(-1.0, "sel_m1")   # ACT |d| streams
        sel_p1 = ones_pair(1.0, "sel_p1")    # box streams

        eps_t = singles.tile([128, 1], F32)
        nc.vector.memset(eps_t[:, :], BN_EPS)

        w_sb = singles.tile([128, CO * 9], F32)     # w_sb[(g,ci), co*9+tap]
        w_src = w.rearrange("co ci kh kw -> ci co (kh kw)")
        nc.scalar.dma_start(
            out=w_sb[0:64, :].rearrange("p (co t) -> p co t", t=9), in_=w_src)
        nc.scalar.dma_start(
            out=w_sb[64:128, :].rearrange("p (co t) -> p co t", t=9), in_=w_src)
        neg_w_sb = singles.tile([128, CO * 9], F32)
        nc.vector.tensor_scalar(
            out=neg_w_sb[:, :], in0=w_sb[:, :], scalar1=-1.0, scalar2=None,
            op0=mybir.AluOpType.mult)

        # stage-2 targets: row = co*16 + g*8 + h*2 + j (n-major per co)
        cs_rl = spool.tile([128, HW], F32)      # S
        box_rl = spool.tile([128, HW], F32)     # Box broadcast per co
        conv_d = dpool.tile([CO, 2, N_HALVES, JPH, HW], F32)
        box_d = dpool.tile([2, N_HALVES, JPH, HW], F32)

        def emit_xload(h):
            x_st = xspool.tile([128, JPH, PADH, PADW], F32, tag="xst",
                               name=f"xst{h}")
            if h < 3:   # borders stay zero across 3-deep buffer reuse
                nc.vector.memset(x_st[:, :, 0, :], 0.0)
                nc.vector.memset(x_st[:, :, PADH - 1, :], 0.0)
                nc.vector.memset(x_st[:, :, 1:PADH - 1, 0:1], 0.0)
                nc.vector.memset(x_st[:, :, 1:PADH - 1, H + 1:PADW], 0.0)
            for g in range(2):
                for jj in range(JPH):
                    eng = nc.sync if (g + jj) % 2 == 0 else nc.scalar
                    eng.dma_start(
                        out=x_st[g * 64:(g + 1) * 64, jj, 1:H + 1, 1:W + 1],
                        in_=x[g * 8 + h * JPH + jj])
            return x_st

        def emit_cast0(h, x_st):
            xb0 = xbpool.tile([128, JPH, PADH, PADW], BF16, tag="xb0",
                              name=f"xb0_{h}")
            nc.vector.tensor_copy(
                out=xb0.rearrange("p a h w -> p (a h w)"),
                in_=x_st.rearrange("p a h w -> p (a h w)"))
            return xb0

        def emit_cast1(h, x_st):
            xb1 = xbpool.tile([128, JPH, PADH, PADW], BF16, tag="xb1",
                              name=f"xb1_{h}")
            nc.scalar.copy(
                out=xb1[:, :, :, 0:PADW - 2],
                in_=x_st[:, :, :, 1:PADW - 1])
            return xb1

        # ---- stage 1 ----
        pendingB = None         # deferred slot-B evacuation from prior half
        xs0 = emit_xload(0)
        xbs = {0: [emit_cast0(0, xs0), emit_cast1(0, xs0)]}
        for half in range(N_HALVES):
            xb0, xb1 = xbs.pop(half)

            def tap_src(tap):
                kh, kw = divmod(tap, 3)
                if kw == 1 and tap not in ACT_TAPS:
                    return xb1, kh, 0        # shifted copy, aligned
                return xb0, kh, kw

            psA = pspool.tile([128, TCOLS], F32, tag="ps", name=f"psA{half}")
            psB = pspool.tile([128, TCOLS], F32, tag="ps", name=f"psB{half}")

            def emit_evac(ps, co0, h, slot):
                # evacuate, bounce via DRAM, reload into the stage-2 rows
                scr = scpool.tile([128, TCOLS], F32, tag="scr",
                                  name=f"scr{h}_{slot}")
                nc.scalar.copy(scr[:, :], ps[:, :])
                for cl in range(4):
                    co = co0 + cl
                    nc.sync.dma_start(
                        out=conv_d[co, :, h, :, :],
                        in_=scr[32 * cl:32 * cl + 2, :].rearrange(
                            "p (a hw) -> p a hw", hw=HW))
                    if h == N_HALVES - 1:
                        nc.sync.dma_start(
                            out=cs_rl[co * 16:(co + 1) * 16, :],
                            in_=conv_d[co].rearrange("g h j w -> (g h j) w"))

            def emit_box_mms(ps, h):
                # box streams: block b at position b (4-way concurrent)
                box_taps = [t for t in range(9) if t not in ACT_TAPS]
                for bi, tap in enumerate(box_taps):
                    src_t, skh, skw = tap_src(tap)
                    for b in range(NB):
                        a, hb = divmod(b, 2)
                        rhs = src_t[:, a, skh + hb * 16:skh + hb * 16 + 16,
                                    skw:skw + W]
                        nc.tensor.matmul(
                            ps[32 * b:32 * b + 2, b * 512:(b + 1) * 512],
                            lhsT=sel_p1[:, :], rhs=rhs,
                            start=(bi == 0), stop=(bi == len(box_taps) - 1),
                            tile_position=(0, 32 * b))

            def emit_box_evac(ps, h):
                scb = sbpool.tile([2, TCOLS], F32, tag="scb",
                                  name=f"scb{h}")
                for b in range(NB):
                    nc.scalar.copy(scb[:, b * 512:(b + 1) * 512],
                                   ps[32 * b:32 * b + 2, b * 512:(b + 1) * 512])
                nc.sync.dma_start(
                    out=box_d[:, h, :, :],
                    in_=scb.rearrange("p (a hw) -> p a hw", hw=HW))
                if h == N_HALVES - 1:
                    for c2 in range(CO):
                        nc.sync.dma_start(
                            out=box_rl[c2 * 16:(c2 + 1) * 16, :],
                            in_=box_d.rearrange("g h j w -> (g h j) w"))

            def emit_taps(ps, co0, deferred):
                """deferred: {ti: fn} emitted after tap-group ti's MMs."""
                for ti, tap in enumerate(TAP_ORDER):
                    src_t, skh, skw = tap_src(tap)
                    on_act = tap in ACT_TAPS
                    sel = sel_m1 if on_act else sel_m2
                    ts = []
                    for cl in range(4):
                        co = co0 + cl
                        k = co * 9 + tap
                        t = tpool.tile([128, JPH, H, W], BF16, tag="t",
                                       name=f"t{half}_{co0}_{ti}_{cl}")
                        src = src_t[:, :, skh:skh + H, skw:skw + W]
                        if on_act:
                            nc.scalar.activation(
                                out=t[:, :, :, :], in_=src,
                                func=mybir.ActivationFunctionType.Abs,
                                bias=neg_w_sb[:, k:k + 1], scale=1.0)
                        else:
                            nc.vector.tensor_scalar(
                                out=t[:, :, :, :], in0=src,
                                scalar1=w_sb[:, k:k + 1], scalar2=0.0,
                                op0=mybir.AluOpType.subtract,
                                op1=mybir.AluOpType.max)
                        ts.append(t.rearrange("p a h w -> p (a h w)"))
                    for b in range(NB):
                        for cl in range(4):
                            nc.tensor.matmul(
                                ps[32 * cl:32 * cl + 2,
                                   b * 512:(b + 1) * 512],
                                lhsT=sel[:, :],
                                rhs=ts[cl][:, b * 512:(b + 1) * 512],
                                start=(ti == 0), stop=(ti == 8),
                                tile_position=(0, 32 * cl))
                    fn = deferred.get(ti)
                    if fn is not None:
                        fn()

            # prefetch next half's x (DMA queue) + defer its casts into tapsA
            defA = {}
            if pendingB is not None:
                defA[0] = pendingB
            if half + 1 < N_HALVES:
                xs_n = emit_xload(half + 1)
                xbs[half + 1] = [None, None]

                def mk0(h1, xst):
                    def fn():
                        xbs[h1][0] = emit_cast0(h1, xst)
                    return fn

                def mk1(h1, xst):
                    def fn():
                        xbs[h1][1] = emit_cast1(h1, xst)
                    return fn
                defA[2] = mk0(half + 1, xs_n)
                defA[3] = mk1(half + 1, xs_n)
            emit_taps(psA, 0, defA)
            # tapsB: evacA after group 1, box MMs after group 2, box evac
            # after group 6 (box MMs done on PE by then)
            defB = {
                1: lambda: emit_evac(psA, 0, half, 0),
                2: lambda: emit_box_mms(psA, half),
                6: lambda: emit_box_evac(psA, half),
            }
            emit_taps(psB, 4, defB)
            if half == N_HALVES - 1:
                emit_evac(psB, 4, half, 1)
            else:
                pendingB = (lambda ps=psB, h=half:
                            emit_evac(ps, 4, h, 1))

        # ---- stage 2: BN stats + affine + relu ----
        selcor = singles.tile([128, 128], F32)      # replicated stats selector
        nc.sync.dma_start(out=selcor[:, :], in_=selcor_in[:, :])
        selcor_r = singles.tile([128, 128], BF16)
        nc.vector.tensor_copy(out=selcor_r[:, :], in_=selcor[:, :])
        gam = singles.tile([128, 1], F32)
        bet = singles.tile([128, 1], F32)
        for co in range(CO):
            nc.sync.dma_start(out=gam[co * 16:(co + 1) * 16, :],
                              in_=gamma[co:co + 1].partition_broadcast(16))
            nc.sync.dma_start(out=bet[co * 16:(co + 1) * 16, :],
                              in_=beta[co:co + 1].partition_broadcast(16))
        conv_rl = spool.tile([128, HW], F32)    # true conv output (+SwD shift)
        nc.vector.tensor_add(conv_rl[:, :], cs_rl[:, :], box_rl[:, :])
        if debug:
            nc.sync.dma_start(out=dbg_cs[:, :], in_=cs_rl[:, :])
            nc.sync.dma_start(out=dbg_box[:, :], in_=box_rl[:, :])

        # stats: mean via replicated-selector matmul, then centered var
        conv_r = spool.tile([128, HW], BF16)
        nc.vector.tensor_copy(out=conv_r[:, :], in_=conv_rl[:, :])
        ps1 = pspool.tile([128, 512], F32, tag="ps")
        ps1b = pspool.tile([128, 512], F32, tag="ps")
        nc.tensor.matmul(ps1[:, :], lhsT=selcor_r[:, :], rhs=conv_r[:, 0:512],
                         start=True, stop=True)
        nc.tensor.matmul(ps1b[:, :], lhsT=selcor_r[:, :], rhs=conv_r[:, 512:HW],
                         start=True, stop=True)
        s1 = spool.tile([128, 1], F32)
        s1b = spool.tile([128, 1], F32)
        nc.vector.tensor_reduce(out=s1[:, :], in_=ps1[:, :],
                                axis=mybir.AxisListType.X, op=mybir.AluOpType.add)
        nc.vector.tensor_reduce(out=s1b[:, :], in_=ps1b[:, :],
                                axis=mybir.AxisListType.X, op=mybir.AluOpType.add)
        inv_n = 1.0 / (N * HW)
        mean = spool.tile([128, 1], F32)
        nc.vector.tensor_scalar(out=mean[:, :], in0=s1[:, :], scalar1=s1b[:, :],
                                scalar2=inv_n, op0=mybir.AluOpType.add,
                                op1=mybir.AluOpType.mult)
        # centered square -> variance without cancellation
        dctr = spool.tile([128, HW], F32)
        nc.vector.tensor_scalar(out=dctr[:, :], in0=conv_rl[:, :],
                                scalar1=mean[:, :], scalar2=None,
                                op0=mybir.AluOpType.subtract)
        sq = spool.tile([128, HW], BF16)
        nc.scalar.activation(out=sq[:, :], in_=dctr[:, :],
                             func=mybir.ActivationFunctionType.Square)
        ps2 = pspool.tile([128, 512], F32, tag="ps")
        ps2b = pspool.tile([128, 512], F32, tag="ps")
        nc.tensor.matmul(ps2[:, :], lhsT=selcor_r[:, :], rhs=sq[:, 0:512],
                         start=True, stop=True)
        nc.tensor.matmul(ps2b[:, :], lhsT=selcor_r[:, :], rhs=sq[:, 512:HW],
                         start=True, stop=True)
        s2 = spool.tile([128, 1], F32)
        s2b = spool.tile([128, 1], F32)
        nc.vector.tensor_reduce(out=s2[:, :], in_=ps2[:, :],
                                axis=mybir.AxisListType.X, op=mybir.AluOpType.add)
        nc.vector.tensor_reduce(out=s2b[:, :], in_=ps2b[:, :],
                                axis=mybir.AxisListType.X, op=mybir.AluOpType.add)
        var = spool.tile([128, 1], F32)
        nc.vector.tensor_scalar(out=var[:, :], in0=s2[:, :], scalar1=s2b[:, :],
                                scalar2=inv_n, op0=mybir.AluOpType.add,
                                op1=mybir.AluOpType.mult)
        std = spool.tile([128, 1], F32)
        nc.scalar.activation(out=std[:, :], in_=var[:, :],
                             func=mybir.ActivationFunctionType.Sqrt,
                             bias=eps_t[:, :], scale=1.0)
        rstd = spool.tile([128, 1], F32)
        nc.vector.reciprocal(out=rstd[:, :], in_=std[:, :])
        a_t = spool.tile([128, 1], F32)
        nc.vector.tensor_mul(a_t[:, :], gam[:, :], rstd[:, :])
        ma = spool.tile([128, 1], F32)
        nc.vector.tensor_mul(ma[:, :], mean[:, :], a_t[:, :])
        b_t = spool.tile([128, 1], F32)
        nc.vector.tensor_sub(b_t[:, :], bet[:, :], ma[:, :])

        outt = spool.tile([128, HW], F32)
        nc.scalar.activation(out=outt[:, :], in_=conv_rl[:, :],
                             func=mybir.ActivationFunctionType.Relu,
                             bias=b_t[:, :], scale=a_t[:, :])
        out_r = out.rearrange("n co h w -> co n (h w)")
        for co in range(CO):
            eng = nc.sync if co % 2 == 0 else nc.scalar
            eng.dma_start(out=out_r[co], in_=outt[co * 16:(co + 1) * 16, :])

    split_multiwaits(nc)
    return nc


def make_in_maps(x, weight, gamma, beta):
    x = np.ascontiguousarray(x, dtype=np.float32)
    weight = np.ascontiguousarray(weight, dtype=np.float32)
    gamma = np.ascontiguousarray(gamma, dtype=np.float32)
    beta = np.ascontiguousarray(beta, dtype=np.float32)
    selcor = np.zeros((128, 128), np.float32)
    for c in range(CO):
        selcor[c * 16:(c + 1) * 16, c * 16:(c + 1) * 16] = 1.0
    maps = []
    for c in range(8):
        sl = slice(c * CO, (c + 1) * CO)
        maps.append({
            "x": x,
            "w": np.ascontiguousarray(weight[sl]),
            "gamma": np.ascontiguousarray(gamma[sl]),
            "beta": np.ascontiguousarray(beta[sl]),
            "selcor": selcor,
        })
    return maps


def assemble(results):
    return np.concatenate([r["out"] for r in results], axis=1)


# ---------------------------------------------------------------------------
# Harness entry point: full inputs in, full output out.
# Sharding: output channels co split 8 ways (8 channels per NeuronCore);
# BN statistics are over the full batch, which each core owns for its
# channels, so no collectives are needed.
# ---------------------------------------------------------------------------
from concourse.bass_utils import run_bass_kernel_spmd

_NC_CACHE = None


def _get_nc():
    global _NC_CACHE
    if _NC_CACHE is None:
        _NC_CACHE = build_nc()
    return _NC_CACHE


def kernel(x, weight, gamma, beta):
    nc = _get_nc()
    in_maps = make_in_maps(np.asarray(x), np.asarray(weight),
                           np.asarray(gamma), np.asarray(beta))
    res = run_bass_kernel_spmd(nc, in_maps, core_ids=list(range(8)))
    return assemble(res.results)


# revision 22
# speedup vs baseline: 1.0544x; 1.0544x over previous
"""AdderNet BasicBlock kernel for Trainium2, co-sharded across 8 cores.

Per core (co-shard CO=8 of 64 output channels):
  conv[co,n,p] = -sum_{ci,kh,kw} |x[n,ci,p+k-1] - w[co,ci,kh,kw]|   (pad=1)
  BN train-mode over (n,h,w) per co, then ReLU.

Formulation (v2):
  Taps are split between engines:
    ACT taps (2 of 9): |d| = Abs(x + (-w)) directly  -> PE weight -1
    DVE taps (7 of 9): relu(x-w) = tensor_scalar(sub, max 0), bf16 4x mode
        |d| = 2*relu(d) - d            -> PE weight -2, Box correction +1
  conv = -(sum_ACT |d| + 2*sum_DVE relu(d)) + Box_D - SwD
  SwD is a per-channel constant -> dropped (BN is shift-invariant per channel).

PE: 4-way column tiling. Position j holds co j (tile A) / co 4+j (tile B);
ones-reduce over 128 partitions (2 image groups x 64 ci), 512-col blocks,
PSUM accumulates 9 taps. Box (7 DVE taps over x) streams into tile A after
its evacuation, at a per-half rotating position.

BN stage 2 as before: bounce conv to DRAM, reload as [(co,n), hw],
replicated-selector matmul stats, fused affine+relu on ACT.
"""
from contextlib import ExitStack

import numpy as np

import concourse.bass as bass
import concourse.tile as tile
import concourse.mybir as mybir

F32 = mybir.dt.float32
BF16 = mybir.dt.bfloat16
BN_EPS = 1e-5

N, CI, H, W = 16, 64, 32, 32
CO = 8          # output channels per core
HW = H * W      # 1024
PADH, PADW = H + 2, W + 4   # 34 x 36 (2 extra zero cols: alignment + even dims)

N_HALVES = 4
JPH = 2                      # images per group per half
TCOLS = JPH * HW             # 2048 free-dim per stream
NB = TCOLS // 512            # 512-col matmul blocks per stream

ACT_TAPS = (1, 7)            # (0,1),(2,1): kw=1 taps go to ACT (Abs direct)
# interleave ACT taps for engine overlap; DVE taps use xb0 (kw even) or xb1
TAP_ORDER = (0, 2, 1, 3, 4, 7, 5, 6, 8)


def split_multiwaits(nc, max_waits=1):
    """This container's walrus rejects >1 semaphore wait per instruction.
    Hoist extras into standalone NoOps on the same (in-order) engine."""
    n_split = 0
    for f in nc.m.functions:
        for b in f.blocks:
            insts = list(b.instructions)
            changed = False
            new = []
            for inst in insts:
                si = inst.sync_info
                waits = list(si.on_wait) if si and si.on_wait else []
                if len(waits) > max_waits:
                    changed = True
                    n_split += 1
                    for w in waits[: len(waits) - max_waits]:
                        new.append(mybir.InstNoOp(
                            name=nc.get_next_instruction_name(),
                            engine=inst.engine, ins=[], outs=[],
                            sync_info=mybir.SyncInfo(on_wait=[w], on_update=[]),
                        ))
                    inst.sync_info = mybir.SyncInfo(
                        on_wait=waits[len(waits) - max_waits:],
                        on_update=list(si.on_update) if si.on_update else [],
                    )
                new.append(inst)
            if changed:
                b.instructions = new
    return n_split


def build_nc(t_bufs=20, debug=False):
    """One core's SPMD program."""
    nc = bass.Bass()
    x = nc.declare_dram_parameter("x", [N, CI, H, W], F32, isOutput=False)
    w = nc.declare_dram_parameter("w", [CO, CI, 3, 3], F32, isOutput=False)
    gamma = nc.declare_dram_parameter("gamma", [CO], F32, isOutput=False)
    beta = nc.declare_dram_parameter("beta", [CO], F32, isOutput=False)
    selcor_in = nc.declare_dram_parameter("selcor", [128, 128], F32,
                                          isOutput=False)
    out = nc.declare_dram_parameter("out", [N, CO, H, W], F32, isOutput=True)
    if debug:
        dbg_cs = nc.declare_dram_parameter("dbg_cs", [128, HW], F32,
                                           isOutput=True)
        dbg_box = nc.declare_dram_parameter("dbg_box", [128, HW], F32,
                                            isOutput=True)

    with tile.TileContext(nc) as tc, ExitStack() as ctx:
        singles = ctx.enter_context(tc.tile_pool(name="singles", bufs=1))
        xspool = ctx.enter_context(tc.tile_pool(name="xstage", bufs=3))
        xbpool = ctx.enter_context(tc.tile_pool(name="xb", bufs=3))
        tpool = ctx.enter_context(tc.tile_pool(name="tpool", bufs=t_bufs))
        scpool = ctx.enter_context(tc.tile_pool(name="scr", bufs=2))
        sbpool = ctx.enter_context(tc.tile_pool(name="scrbox", bufs=2))
        pspool = ctx.enter_context(tc.tile_pool(name="psum", bufs=2,
                                                space="PSUM"))
        spool = ctx.enter_context(tc.tile_pool(name="stage2", bufs=1))
        dpool = ctx.enter_context(tc.tile_pool(name="dram", bufs=1,
                                               space="DRAM"))

        # ---- constants ----
        def ones_pair(val, nm):
            t = singles.tile([128, 2], BF16, name=nm)
            nc.vector.memset(t[:, :], 0.0)
            nc.vector.memset(t[0:64, 0:1], val)
            nc.vector.memset(t[64:128, 1:2], val)
            return t
        sel_m2 = ones_pair(-2.0, "sel_m2")   # DVE relu streams
        sel_m1 = ones_pair(-1.0, "sel_m1")   # ACT |d| streams
        sel_p1 = ones_pair(1.0, "sel_p1")    # box streams

        eps_t = singles.tile([128, 1], F32)
        nc.vector.memset(eps_t[:, :], BN_EPS)

        w_sb = singles.tile([128, CO * 9], F32)     # w_sb[(g,ci), co*9+tap]
        w_src = w.rearrange("co ci kh kw -> ci co (kh kw)")
        nc.sync.dma_start(
            out=w_sb[0:64, :].rearrange("p (co t) -> p co t", t=9), in_=w_src)
        nc.sync.dma_start(
            out=w_sb[64:128, :].rearrange("p (co t) -> p co t", t=9), in_=w_src)
        neg_w_sb = singles.tile([128, CO * 9], F32)
        nc.vector.tensor_scalar(
            out=neg_w_sb[:, :], in0=w_sb[:, :], scalar1=-1.0, scalar2=None,
            op0=mybir.AluOpType.mult)

        # stage-2 targets: row = co*16 + g*8 + h*2 + j (n-major per co)
        cs_rl = spool.tile([128, HW], F32)      # S
        box_rl = spool.tile([128, HW], F32)     # Box broadcast per co
        conv_d = dpool.tile([CO, 2, N_HALVES, JPH, HW], F32)
        box_d = dpool.tile([2, N_HALVES, JPH, HW], F32)

        def emit_xload(h):
            x_st = xspool.tile([128, JPH, PADH, PADW], F32, tag="xst",
                               name=f"xst{h}")
            if h < 3:   # borders stay zero across 3-deep buffer reuse
                nc.vector.memset(x_st[:, :, 0, :], 0.0)
                nc.vector.memset(x_st[:, :, PADH - 1, :], 0.0)
                nc.vector.memset(x_st[:, :, 1:PADH - 1, 0:1], 0.0)
                nc.vector.memset(x_st[:, :, 1:PADH - 1, H + 1:PADW], 0.0)
            for g in range(2):
                for jj in range(JPH):
                    nc.sync.dma_start(
                        out=x_st[g * 64:(g + 1) * 64, jj, 1:H + 1, 1:W + 1],
                        in_=x[g * 8 + h * JPH + jj])
            return x_st

        def emit_cast0(h, x_st):
            xb0 = xbpool.tile([128, JPH, PADH, PADW], BF16, tag="xb0",
                              name=f"xb0_{h}")
            nc.vector.tensor_copy(
                out=xb0.rearrange("p a h w -> p (a h w)"),
                in_=x_st.rearrange("p a h w -> p (a h w)"))
            return xb0

        def emit_cast1(h, x_st):
            xb1 = xbpool.tile([128, JPH, PADH, PADW], BF16, tag="xb1",
                              name=f"xb1_{h}")
            nc.scalar.copy(
                out=xb1[:, :, :, 0:PADW - 2],
                in_=x_st[:, :, :, 1:PADW - 1])
            return xb1

        # ---- stage 1 ----
        pendingB = None         # deferred slot-B evacuation from prior half
        xs0 = emit_xload(0)
        xbs = {0: [emit_cast0(0, xs0), emit_cast1(0, xs0)]}
        for half in range(N_HALVES):
            xb0, xb1 = xbs.pop(half)

            def tap_src(tap):
                kh, kw = divmod(tap, 3)
                if kw == 1 and tap not in ACT_TAPS:
                    return xb1, kh, 0        # shifted copy, aligned
                return xb0, kh, kw

            psA = pspool.tile([128, TCOLS], F32, tag="ps", name=f"psA{half}")
            psB = pspool.tile([128, TCOLS], F32, tag="ps", name=f"psB{half}")

            def emit_evac(ps, co0, h, slot):
                # evacuate, bounce via DRAM, reload into the stage-2 rows
                scr = scpool.tile([128, TCOLS], F32, tag="scr",
                                  name=f"scr{h}_{slot}")
                nc.scalar.copy(scr[:, :], ps[:, :])
                for cl in range(4):
                    co = co0 + cl
                    nc.sync.dma_start(
                        out=conv_d[co, :, h, :, :],
                        in_=scr[32 * cl:32 * cl + 2, :].rearrange(
                            "p (a hw) -> p a hw", hw=HW))
                    if h == N_HALVES - 1:
                        nc.sync.dma_start(
                            out=cs_rl[co * 16:(co + 1) * 16, :],
                            in_=conv_d[co].rearrange("g h j w -> (g h j) w"))

            def emit_box_mms(ps, h):
                # box streams: block b at position b (4-way concurrent)
                box_taps = [t for t in range(9) if t not in ACT_TAPS]
                for bi, tap in enumerate(box_taps):
                    src_t, skh, skw = tap_src(tap)
                    for b in range(NB):
                        a, hb = divmod(b, 2)
                        rhs = src_t[:, a, skh + hb * 16:skh + hb * 16 + 16,
                                    skw:skw + W]
                        nc.tensor.matmul(
                            ps[32 * b:32 * b + 2, b * 512:(b + 1) * 512],
                            lhsT=sel_p1[:, :], rhs=rhs,
                            start=(bi == 0), stop=(bi == len(box_taps) - 1),
                            tile_position=(0, 32 * b))

            def emit_box_evac(ps, h):
                scb = sbpool.tile([2, TCOLS], F32, tag="scb",
                                  name=f"scb{h}")
                for b in range(NB):
                    nc.scalar.copy(scb[:, b * 512:(b + 1) * 512],
                                   ps[32 * b:32 * b + 2, b * 512:(b + 1) * 512])
                nc.sync.dma_start(
                    out=box_d[:, h, :, :],
                    in_=scb.rearrange("p (a hw) -> p a hw", hw=HW))
                if h == N_HALVES - 1:
                    for c2 in range(CO):
                        nc.sync.dma_start(
                            out=box_rl[c2 * 16:(c2 + 1) * 16, :],
                            in_=box_d.rearrange("g h j w -> (g h j) w"))

            def emit_taps(ps, co0, deferred):
                """deferred: {ti: fn} emitted after tap-group ti's MMs."""
                for ti, tap in enumerate(TAP_ORDER):
                    src_t, skh, skw = tap_src(tap)
                    on_act = tap in ACT_TAPS
                    sel = sel_m1 if on_act else sel_m2
                    ts = []
                    for cl in range(4):
                        co = co0 + cl
                        k = co * 9 + tap
                        t = tpool.tile([128, JPH, H, W], BF16, tag="t",
                                       name=f"t{half}_{co0}_{ti}_{cl}")
                        src = src_t[:, :, skh:skh + H, skw:skw + W]
                        if on_act:
                            nc.scalar.activation(
                                out=t[:, :, :, :], in_=src,
                                func=mybir.ActivationFunctionType.Abs,
                                bias=neg_w_sb[:, k:k + 1], scale=1.0)
                        else:
                            nc.vector.tensor_scalar(
                                out=t[:, :, :, :], in0=src,
                                scalar1=w_sb[:, k:k + 1], scalar2=0.0,
                                op0=mybir.AluOpType.subtract,
                                op1=mybir.AluOpType.max)
                        ts.append(t.rearrange("p a h w -> p (a h w)"))
                    for b in range(NB):
                        for cl in range(4):
                            nc.tensor.matmul(
                                ps[32 * cl:32 * cl + 2,
                                   b * 512:(b + 1) * 512],
                                lhsT=sel[:, :],
                                rhs=ts[cl][:, b * 512:(b + 1) * 512],
                                start=(ti == 0), stop=(ti == 8),
                                tile_position=(0, 32 * cl))
                    fn = deferred.get(ti)
                    if fn is not None:
                        fn()

            # prefetch next half's x (DMA queue) + defer its casts into tapsA
            defA = {}
            if pendingB is not None:
                defA[0] = pendingB
            if half + 1 < N_HALVES:
                xs_n = emit_xload(half + 1)
                xbs[half + 1] = [None, None]

                def mk0(h1, xst):
                    def fn():
                        xbs[h1][0] = emit_cast0(h1, xst)
                    return fn

                def mk1(h1, xst):
                    def fn():
                        xbs[h1][1] = emit_cast1(h1, xst)
                    return fn
                defA[2] = mk0(half + 1, xs_n)
                defA[3] = mk1(half + 1, xs_n)
            emit_taps(psA, 0, defA)
            # tapsB: evacA after group 1, box MMs after group 2, box evac
            # after group 6 (box MMs done on PE by then)
            defB = {
                1: lambda: emit_evac(psA, 0, half, 0),
                2: lambda: emit_box_mms(psA, half),
                6: lambda: emit_box_evac(psA, half),
            }
            emit_taps(psB, 4, defB)
            if half == N_HALVES - 1:
                emit_evac(psB, 4, half, 1)
            else:
                pendingB = (lambda ps=psB, h=half:
                            emit_evac(ps, 4, h, 1))

        # ---- stage 2: BN stats + affine + relu ----
        selcor = singles.tile([128, 128], F32)      # replicated stats selector
        nc.sync.dma_start(out=selcor[:, :], in_=selcor_in[:, :])
        selcor_r = singles.tile([128, 128], BF16)
        nc.vector.tensor_copy(out=selcor_r[:, :], in_=selcor[:, :])
        gam = singles.tile([128, 1], F32)
        bet = singles.tile([128, 1], F32)
        for co in range(CO):
            nc.sync.dma_start(out=gam[co * 16:(co + 1) * 16, :],
                              in_=gamma[co:co + 1].partition_broadcast(16))
            nc.sync.dma_start(out=bet[co * 16:(co + 1) * 16, :],
                              in_=beta[co:co + 1].partition_broadcast(16))
        conv_rl = spool.tile([128, HW], F32)    # true conv output (+SwD shift)
        nc.vector.tensor_add(conv_rl[:, :], cs_rl[:, :], box_rl[:, :])
        if debug:
            nc.sync.dma_start(out=dbg_cs[:, :], in_=cs_rl[:, :])
            nc.sync.dma_start(out=dbg_box[:, :], in_=box_rl[:, :])

        # stats: mean via replicated-selector matmul, then centered var
        conv_r = spool.tile([128, HW], BF16)
        nc.vector.tensor_copy(out=conv_r[:, :], in_=conv_rl[:, :])
        ps1 = pspool.tile([128, 512], F32, tag="ps")
        ps1b = pspool.tile([128, 512], F32, tag="ps")
        nc.tensor.matmul(ps1[:, :], lhsT=selcor_r[:, :], rhs=conv_r[:, 0:512],
                         start=True, stop=True)
        nc.tensor.matmul(ps1b[:, :], lhsT=selcor_r[:, :], rhs=conv_r[:, 512:HW],
                         start=True, stop=True)
        s1 = spool.tile([128, 1], F32)
        s1b = spool.tile([128, 1], F32)
        nc.vector.tensor_reduce(out=s1[:, :], in_=ps1[:, :],
                                axis=mybir.AxisListType.X, op=mybir.AluOpType.add)
        nc.vector.tensor_reduce(out=s1b[:, :], in_=ps1b[:, :],
                                axis=mybir.AxisListType.X, op=mybir.AluOpType.add)
        inv_n = 1.0 / (N * HW)
        mean = spool.tile([128, 1], F32)
        nc.vector.tensor_scalar(out=mean[:, :], in0=s1[:, :], scalar1=s1b[:, :],
                                scalar2=inv_n, op0=mybir.AluOpType.add,
                                op1=mybir.AluOpType.mult)
        # centered square -> variance without cancellation
        dctr = spool.tile([128, HW], F32)
        nc.vector.tensor_scalar(out=dctr[:, :], in0=conv_rl[:, :],
                                scalar1=mean[:, :], scalar2=None,
                                op0=mybir.AluOpType.subtract)
        sq = spool.tile([128, HW], BF16)
        nc.scalar.activation(out=sq[:, :], in_=dctr[:, :],
                             func=mybir.ActivationFunctionType.Square)
        ps2 = pspool.tile([128, 512], F32, tag="ps")
        ps2b = pspool.tile([128, 512], F32, tag="ps")
        nc.tensor.matmul(ps2[:, :], lhsT=selcor_r[:, :], rhs=sq[:, 0:512],
                         start=True, stop=True)
        nc.tensor.matmul(ps2b[:, :], lhsT=selcor_r[:, :], rhs=sq[:, 512:HW],
                         start=True, stop=True)
        s2 = spool.tile([128, 1], F32)
        s2b = spool.tile([128, 1], F32)
        nc.vector.tensor_reduce(out=s2[:, :], in_=ps2[:, :],
                                axis=mybir.AxisListType.X, op=mybir.AluOpType.add)
        nc.vector.tensor_reduce(out=s2b[:, :], in_=ps2b[:, :],
                                axis=mybir.AxisListType.X, op=mybir.AluOpType.add)
        var = spool.tile([128, 1], F32)
        nc.vector.tensor_scalar(out=var[:, :], in0=s2[:, :], scalar1=s2b[:, :],
                                scalar2=inv_n, op0=mybir.AluOpType.add,
                                op1=mybir.AluOpType.mult)
        std = spool.tile([128, 1], F32)
        nc.scalar.activation(out=std[:, :], in_=var[:, :],
                             func=mybir.ActivationFunctionType.Sqrt,
                             bias=eps_t[:, :], scale=1.0)
        rstd = spool.tile([128, 1], F32)
        nc.vector.reciprocal(out=rstd[:, :], in_=std[:, :])
        a_t = spool.tile([128, 1], F32)
        nc.vector.tensor_mul(a_t[:, :], gam[:, :], rstd[:, :])
        ma = spool.tile([128, 1], F32)
        nc.vector.tensor_mul(ma[:, :], mean[:, :], a_t[:, :])
        b_t = spool.tile([128, 1], F32)
        nc.vector.tensor_sub(b_t[:, :], bet[:, :], ma[:, :])

        outt = spool.tile([128, HW], F32)
        nc.scalar.activation(out=outt[:, :], in_=conv_rl[:, :],
                             func=mybir.ActivationFunctionType.Relu,
                             bias=b_t[:, :], scale=a_t[:, :])
        out_r = out.rearrange("n co h w -> co n (h w)")
        for co in range(CO):
            nc.sync.dma_start(out=out_r[co], in_=outt[co * 16:(co + 1) * 16, :])

    split_multiwaits(nc)
    return nc


def make_in_maps(x, weight, gamma, beta):
    x = np.ascontiguousarray(x, dtype=np.float32)
    weight = np.ascontiguousarray(weight, dtype=np.float32)
    gamma = np.ascontiguousarray(gamma, dtype=np.float32)
    beta = np.ascontiguousarray(beta, dtype=np.float32)
    selcor = np.zeros((128, 128), np.float32)
    for c in range(CO):
        selcor[c * 16:(c + 1) * 16, c * 16:(c + 1) * 16] = 1.0
    maps = []
    for c in range(8):
        sl = slice(c * CO, (c + 1) * CO)
        maps.append({
            "x": x,
            "w": np.ascontiguousarray(weight[sl]),
            "gamma": np.ascontiguousarray(gamma[sl]),
            "beta": np.ascontiguousarray(beta[sl]),
            "selcor": selcor,
        })
    return maps


def assemble(results):
    return np.concatenate([r["out"] for r in results], axis=1)


# ---------------------------------------------------------------------------
# Harness entry point: full inputs in, full output out.
# Sharding: output channels co split 8 ways (8 channels per NeuronCore);
# BN statistics are over the full batch, which each core owns for its
# channels, so no collectives are needed.
# ---------------------------------------------------------------------------
from concourse.bass_utils import run_bass_kernel_spmd

_NC_CACHE = None


def _get_nc():
    global _NC_CACHE
    if _NC_CACHE is None:
        _NC_CACHE = build_nc()
    return _NC_CACHE


def kernel(x, weight, gamma, beta):
    nc = _get_nc()
    in_maps = make_in_maps(np.asarray(x), np.asarray(weight),
                           np.asarray(gamma), np.asarray(beta))
    res = run_bass_kernel_spmd(nc, in_maps, core_ids=list(range(8)))
    return assemble(res.results)


# revision 24
# speedup vs baseline: 1.0831x; 1.0273x over previous
"""AdderNet BasicBlock kernel for Trainium2, co-sharded across 8 cores.

Per core (co-shard CO=8 of 64 output channels):
  conv[co,n,p] = -sum_{ci,kh,kw} |x[n,ci,p+k-1] - w[co,ci,kh,kw]|   (pad=1)
  BN train-mode over (n,h,w) per co, then ReLU.

Formulation (v2):
  Taps are split between engines:
    ACT taps (2 of 9): |d| = Abs(x + (-w)) directly  -> PE weight -1
    DVE taps (7 of 9): relu(x-w) = tensor_scalar(sub, max 0), bf16 4x mode
        |d| = 2*relu(d) - d            -> PE weight -2, Box correction +1
  conv = -(sum_ACT |d| + 2*sum_DVE relu(d)) + Box_D - SwD
  SwD is a per-channel constant -> dropped (BN is shift-invariant per channel).

PE: 4-way column tiling. Position j holds co j (tile A) / co 4+j (tile B);
ones-reduce over 128 partitions (2 image groups x 64 ci), 512-col blocks,
PSUM accumulates 9 taps. Box (7 DVE taps over x) streams into tile A after
its evacuation, at a per-half rotating position.

BN stage 2 as before: bounce conv to DRAM, reload as [(co,n), hw],
replicated-selector matmul stats, fused affine+relu on ACT.
"""
from contextlib import ExitStack

import numpy as np

import concourse.bass as bass
import concourse.tile as tile
import concourse.mybir as mybir

F32 = mybir.dt.float32
BF16 = mybir.dt.bfloat16
BN_EPS = 1e-5

N, CI, H, W = 16, 64, 32, 32
CO = 8          # output channels per core
HW = H * W      # 1024
PADH, PADW = H + 2, W + 4   # 34 x 36 (2 extra zero cols: alignment + even dims)

N_HALVES = 4
JPH = 2                      # images per group per half
TCOLS = JPH * HW             # 2048 free-dim per stream
NB = TCOLS // 512            # 512-col matmul blocks per stream

ACT_TAPS = (1, 7)            # (0,1),(2,1): kw=1 taps go to ACT (Abs direct)
# interleave ACT taps for engine overlap; DVE taps use xb0 (kw even) or xb1
TAP_ORDER = (0, 2, 1, 3, 4, 7, 5, 6, 8)


def split_multiwaits(nc, max_waits=1):
    """This container's walrus rejects >1 semaphore wait per instruction.
    Hoist extras into standalone NoOps on the same (in-order) engine."""
    n_split = 0
    for f in nc.m.functions:
        for b in f.blocks:
            insts = list(b.instructions)
            changed = False
            new = []
            for inst in insts:
                si = inst.sync_info
                waits = list(si.on_wait) if si and si.on_wait else []
                if len(waits) > max_waits:
                    changed = True
                    n_split += 1
                    for w in waits[: len(waits) - max_waits]:
                        new.append(mybir.InstNoOp(
                            name=nc.get_next_instruction_name(),
                            engine=inst.engine, ins=[], outs=[],
                            sync_info=mybir.SyncInfo(on_wait=[w], on_update=[]),
                        ))
                    inst.sync_info = mybir.SyncInfo(
                        on_wait=waits[len(waits) - max_waits:],
                        on_update=list(si.on_update) if si.on_update else [],
                    )
                new.append(inst)
            if changed:
                b.instructions = new
    return n_split


def build_nc(t_bufs=22, debug=False):
    """One core's SPMD program."""
    nc = bass.Bass()
    x = nc.declare_dram_parameter("x", [N, CI, H, W], F32, isOutput=False)
    w = nc.declare_dram_parameter("w", [CO, CI, 3, 3], F32, isOutput=False)
    gamma = nc.declare_dram_parameter("gamma", [CO], F32, isOutput=False)
    beta = nc.declare_dram_parameter("beta", [CO], F32, isOutput=False)
    selcor_in = nc.declare_dram_parameter("selcor", [128, 128], F32,
                                          isOutput=False)
    out = nc.declare_dram_parameter("out", [N, CO, H, W], F32, isOutput=True)
    if debug:
        dbg_cs = nc.declare_dram_parameter("dbg_cs", [128, HW], F32,
                                           isOutput=True)
        dbg_box = nc.declare_dram_parameter("dbg_box", [128, HW], F32,
                                            isOutput=True)

    with tile.TileContext(nc) as tc, ExitStack() as ctx:
        singles = ctx.enter_context(tc.tile_pool(name="singles", bufs=1))
        xspool = ctx.enter_context(tc.tile_pool(name="xstage", bufs=3))
        xbpool = ctx.enter_context(tc.tile_pool(name="xb", bufs=3))
        tpool = ctx.enter_context(tc.tile_pool(name="tpool", bufs=t_bufs))
        scpool = ctx.enter_context(tc.tile_pool(name="scr", bufs=2))
        sbpool = ctx.enter_context(tc.tile_pool(name="scrbox", bufs=2))
        pspool = ctx.enter_context(tc.tile_pool(name="psum", bufs=2,
                                                space="PSUM"))
        spool = ctx.enter_context(tc.tile_pool(name="stage2", bufs=1))
        dpool = ctx.enter_context(tc.tile_pool(name="dram", bufs=1,
                                               space="DRAM"))

        # ---- constants ----
        def ones_pair(val, nm):
            t = singles.tile([128, 2], BF16, name=nm)
            nc.vector.memset(t[:, :], 0.0)
            nc.vector.memset(t[0:64, 0:1], val)
            nc.vector.memset(t[64:128, 1:2], val)
            return t
        sel_m2 = ones_pair(-2.0, "sel_m2")   # DVE relu streams
        sel_m1 = ones_pair(-1.0, "sel_m1")   # ACT |d| streams
        sel_p1 = ones_pair(1.0, "sel_p1")    # box streams

        eps_t = singles.tile([128, 1], F32)
        nc.vector.memset(eps_t[:, :], BN_EPS)

        w_sb = singles.tile([128, CO * 9], F32)     # w_sb[(g,ci), co*9+tap]
        w_src = w.rearrange("co ci kh kw -> ci co (kh kw)")
        nc.sync.dma_start(
            out=w_sb[0:64, :].rearrange("p (co t) -> p co t", t=9), in_=w_src)
        nc.sync.dma_start(
            out=w_sb[64:128, :].rearrange("p (co t) -> p co t", t=9), in_=w_src)
        neg_w_sb = singles.tile([128, CO * 9], F32)
        nc.vector.tensor_scalar(
            out=neg_w_sb[:, :], in0=w_sb[:, :], scalar1=-1.0, scalar2=None,
            op0=mybir.AluOpType.mult)

        # stage-2 targets: row = co*16 + g*8 + h*2 + j (n-major per co)
        cs_rl = spool.tile([128, HW], F32)      # S
        box_rl = spool.tile([128, HW], F32)     # Box broadcast per co
        conv_d = dpool.tile([CO, 2, N_HALVES, JPH, HW], F32)
        box_d = dpool.tile([2, N_HALVES, JPH, HW], F32)

        def emit_xload(h):
            x_st = xspool.tile([128, JPH, PADH, PADW], F32, tag="xst",
                               name=f"xst{h}")
            if h < 3:   # borders stay zero across 3-deep buffer reuse
                nc.vector.memset(x_st[:, :, 0, :], 0.0)
                nc.vector.memset(x_st[:, :, PADH - 1, :], 0.0)
                nc.vector.memset(x_st[:, :, 1:PADH - 1, 0:1], 0.0)
                nc.vector.memset(x_st[:, :, 1:PADH - 1, H + 1:PADW], 0.0)
            for g in range(2):
                for jj in range(JPH):
                    nc.sync.dma_start(
                        out=x_st[g * 64:(g + 1) * 64, jj, 1:H + 1, 1:W + 1],
                        in_=x[g * 8 + h * JPH + jj])
            return x_st

        def emit_cast0(h, x_st):
            xb0 = xbpool.tile([128, JPH, PADH, PADW], BF16, tag="xb0",
                              name=f"xb0_{h}")
            nc.vector.tensor_copy(
                out=xb0.rearrange("p a h w -> p (a h w)"),
                in_=x_st.rearrange("p a h w -> p (a h w)"))
            return xb0

        def emit_cast1(h, x_st):
            xb1 = xbpool.tile([128, JPH, PADH, PADW], BF16, tag="xb1",
                              name=f"xb1_{h}")
            nc.scalar.copy(
                out=xb1[:, :, :, 0:PADW - 2],
                in_=x_st[:, :, :, 1:PADW - 1])
            return xb1

        # ---- stage 1 ----
        pendingB = None         # deferred slot-B evacuation from prior half
        xs0 = emit_xload(0)
        xbs = {0: [emit_cast0(0, xs0), emit_cast1(0, xs0)]}
        for half in range(N_HALVES):
            xb0, xb1 = xbs.pop(half)

            def tap_src(tap):
                kh, kw = divmod(tap, 3)
                if kw == 1 and tap not in ACT_TAPS:
                    return xb1, kh, 0        # shifted copy, aligned
                return xb0, kh, kw

            psA = pspool.tile([128, TCOLS], F32, tag="ps", name=f"psA{half}")
            psB = pspool.tile([128, TCOLS], F32, tag="ps", name=f"psB{half}")

            def emit_evac(ps, co0, h, slot):
                # evacuate, bounce via DRAM, reload into the stage-2 rows
                scr = scpool.tile([128, TCOLS], F32, tag="scr",
                                  name=f"scr{h}_{slot}")
                nc.scalar.copy(scr[:, :], ps[:, :])
                for cl in range(4):
                    co = co0 + cl
                    nc.sync.dma_start(
                        out=conv_d[co, :, h, :, :],
                        in_=scr[32 * cl:32 * cl + 2, :].rearrange(
                            "p (a hw) -> p a hw", hw=HW))
                    if h == N_HALVES - 1:
                        nc.sync.dma_start(
                            out=cs_rl[co * 16:(co + 1) * 16, :],
                            in_=conv_d[co].rearrange("g h j w -> (g h j) w"))

            def emit_box_mms(ps, h):
                # box streams: block b at position b (4-way concurrent)
                box_taps = [t for t in range(9) if t not in ACT_TAPS]
                for bi, tap in enumerate(box_taps):
                    src_t, skh, skw = tap_src(tap)
                    for b in range(NB):
                        a, hb = divmod(b, 2)
                        rhs = src_t[:, a, skh + hb * 16:skh + hb * 16 + 16,
                                    skw:skw + W]
                        nc.tensor.matmul(
                            ps[32 * b:32 * b + 2, b * 512:(b + 1) * 512],
                            lhsT=sel_p1[:, :], rhs=rhs,
                            start=(bi == 0), stop=(bi == len(box_taps) - 1),
                            tile_position=(0, 32 * b))

            def emit_box_evac(ps, h):
                scb = sbpool.tile([2, TCOLS], F32, tag="scb",
                                  name=f"scb{h}")
                for b in range(NB):
                    nc.scalar.copy(scb[:, b * 512:(b + 1) * 512],
                                   ps[32 * b:32 * b + 2, b * 512:(b + 1) * 512])
                nc.sync.dma_start(
                    out=box_d[:, h, :, :],
                    in_=scb.rearrange("p (a hw) -> p a hw", hw=HW))
                if h == N_HALVES - 1:
                    for c2 in range(CO):
                        nc.sync.dma_start(
                            out=box_rl[c2 * 16:(c2 + 1) * 16, :],
                            in_=box_d.rearrange("g h j w -> (g h j) w"))

            def emit_taps(ps, co0, deferred):
                """deferred: {ti: fn} emitted after tap-group ti's MMs."""
                for ti, tap in enumerate(TAP_ORDER):
                    src_t, skh, skw = tap_src(tap)
                    on_act = tap in ACT_TAPS
                    sel = sel_m1 if on_act else sel_m2
                    ts = []
                    for cl in range(4):
                        co = co0 + cl
                        k = co * 9 + tap
                        t = tpool.tile([128, JPH, H, W], BF16, tag="t",
                                       name=f"t{half}_{co0}_{ti}_{cl}")
                        src = src_t[:, :, skh:skh + H, skw:skw + W]
                        if on_act:
                            nc.scalar.activation(
                                out=t[:, :, :, :], in_=src,
                                func=mybir.ActivationFunctionType.Abs,
                                bias=neg_w_sb[:, k:k + 1], scale=1.0)
                        else:
                            nc.vector.tensor_scalar(
                                out=t[:, :, :, :], in0=src,
                                scalar1=w_sb[:, k:k + 1], scalar2=0.0,
                                op0=mybir.AluOpType.subtract,
                                op1=mybir.AluOpType.max)
                        ts.append(t.rearrange("p a h w -> p (a h w)"))
                    for b in range(NB):
                        for cl in range(4):
                            nc.tensor.matmul(
                                ps[32 * cl:32 * cl + 2,
                                   b * 512:(b + 1) * 512],
                                lhsT=sel[:, :],
                                rhs=ts[cl][:, b * 512:(b + 1) * 512],
                                start=(ti == 0), stop=(ti == 8),
                                tile_position=(0, 32 * cl))
                    fn = deferred.get(ti)
                    if fn is not None:
                        fn()

            # prefetch next half's x (DMA queue) + defer its casts into tapsA
            defA = {}
            if pendingB is not None:
                defA[0] = pendingB
            if half + 1 < N_HALVES:
                xs_n = emit_xload(half + 1)
                xbs[half + 1] = [None, None]

                def mk0(h1, xst):
                    def fn():
                        xbs[h1][0] = emit_cast0(h1, xst)
                    return fn

                def mk1(h1, xst):
                    def fn():
                        xbs[h1][1] = emit_cast1(h1, xst)
                    return fn
                defA[2] = mk0(half + 1, xs_n)
                defA[3] = mk1(half + 1, xs_n)
            emit_taps(psA, 0, defA)
            # tapsB: evacA after group 1, box MMs after group 2, box evac
            # after group 6 (box MMs done on PE by then)
            defB = {
                1: lambda: emit_evac(psA, 0, half, 0),
                2: lambda: emit_box_mms(psA, half),
                6: lambda: emit_box_evac(psA, half),
            }
            emit_taps(psB, 4, defB)
            if half == N_HALVES - 1:
                emit_evac(psB, 4, half, 1)
            else:
                pendingB = (lambda ps=psB, h=half:
                            emit_evac(ps, 4, h, 1))

        # ---- stage 2: BN stats + affine + relu ----
        selcor = singles.tile([128, 128], F32)      # replicated stats selector
        nc.sync.dma_start(out=selcor[:, :], in_=selcor_in[:, :])
        selcor_r = singles.tile([128, 128], BF16)
        nc.vector.tensor_copy(out=selcor_r[:, :], in_=selcor[:, :])
        gam = singles.tile([128, 1], F32)
        bet = singles.tile([128, 1], F32)
        for co in range(CO):
            nc.sync.dma_start(out=gam[co * 16:(co + 1) * 16, :],
                              in_=gamma[co:co + 1].partition_broadcast(16))
            nc.sync.dma_start(out=bet[co * 16:(co + 1) * 16, :],
                              in_=beta[co:co + 1].partition_broadcast(16))
        conv_rl = spool.tile([128, HW], F32)    # true conv output (+SwD shift)
        nc.vector.tensor_add(conv_rl[:, :], cs_rl[:, :], box_rl[:, :])
        if debug:
            nc.sync.dma_start(out=dbg_cs[:, :], in_=cs_rl[:, :])
            nc.sync.dma_start(out=dbg_box[:, :], in_=box_rl[:, :])

        # stats: mean via replicated-selector matmul, then centered var
        conv_r = spool.tile([128, HW], BF16)
        nc.vector.tensor_copy(out=conv_r[:, :], in_=conv_rl[:, :])
        ps1 = pspool.tile([128, 512], F32, tag="ps")
        ps1b = pspool.tile([128, 512], F32, tag="ps")
        nc.tensor.matmul(ps1[:, :], lhsT=selcor_r[:, :], rhs=conv_r[:, 0:512],
                         start=True, stop=True)
        nc.tensor.matmul(ps1b[:, :], lhsT=selcor_r[:, :], rhs=conv_r[:, 512:HW],
                         start=True, stop=True)
        s1 = spool.tile([128, 1], F32)
        s1b = spool.tile([128, 1], F32)
        nc.vector.tensor_reduce(out=s1[:, :], in_=ps1[:, :],
                                axis=mybir.AxisListType.X, op=mybir.AluOpType.add)
        nc.vector.tensor_reduce(out=s1b[:, :], in_=ps1b[:, :],
                                axis=mybir.AxisListType.X, op=mybir.AluOpType.add)
        inv_n = 1.0 / (N * HW)
        mean = spool.tile([128, 1], F32)
        nc.vector.tensor_scalar(out=mean[:, :], in0=s1[:, :], scalar1=s1b[:, :],
                                scalar2=inv_n, op0=mybir.AluOpType.add,
                                op1=mybir.AluOpType.mult)
        # centered square -> variance without cancellation
        dctr = spool.tile([128, HW], F32)
        nc.vector.tensor_scalar(out=dctr[:, :], in0=conv_rl[:, :],
                                scalar1=mean[:, :], scalar2=None,
                                op0=mybir.AluOpType.subtract)
        sq = spool.tile([128, HW], BF16)
        nc.scalar.activation(out=sq[:, :], in_=dctr[:, :],
                             func=mybir.ActivationFunctionType.Square)
        ps2 = pspool.tile([128, 512], F32, tag="ps")
        ps2b = pspool.tile([128, 512], F32, tag="ps")
        nc.tensor.matmul(ps2[:, :], lhsT=selcor_r[:, :], rhs=sq[:, 0:512],
                         start=True, stop=True)
        nc.tensor.matmul(ps2b[:, :], lhsT=selcor_r[:, :], rhs=sq[:, 512:HW],
                         start=True, stop=True)
        s2 = spool.tile([128, 1], F32)
        s2b = spool.tile([128, 1], F32)
        nc.vector.tensor_reduce(out=s2[:, :], in_=ps2[:, :],
                                axis=mybir.AxisListType.X, op=mybir.AluOpType.add)
        nc.vector.tensor_reduce(out=s2b[:, :], in_=ps2b[:, :],
                                axis=mybir.AxisListType.X, op=mybir.AluOpType.add)
        var = spool.tile([128, 1], F32)
        nc.vector.tensor_scalar(out=var[:, :], in0=s2[:, :], scalar1=s2b[:, :],
                                scalar2=inv_n, op0=mybir.AluOpType.add,
                                op1=mybir.AluOpType.mult)
        std = spool.tile([128, 1], F32)
        nc.scalar.activation(out=std[:, :], in_=var[:, :],
                             func=mybir.ActivationFunctionType.Sqrt,
                             bias=eps_t[:, :], scale=1.0)
        rstd = spool.tile([128, 1], F32)
        nc.vector.reciprocal(out=rstd[:, :], in_=std[:, :])
        a_t = spool.tile([128, 1], F32)
        nc.vector.tensor_mul(a_t[:, :], gam[:, :], rstd[:, :])
        ma = spool.tile([128, 1], F32)
        nc.vector.tensor_mul(ma[:, :], mean[:, :], a_t[:, :])
        b_t = spool.tile([128, 1], F32)
        nc.vector.tensor_sub(b_t[:, :], bet[:, :], ma[:, :])

        outt = spool.tile([128, HW], F32)
        nc.scalar.activation(out=outt[:, :], in_=conv_rl[:, :],
                             func=mybir.ActivationFunctionType.Relu,
                             bias=b_t[:, :], scale=a_t[:, :])
        out_r = out.rearrange("n co h w -> co n (h w)")
        for co in range(CO):
            nc.sync.dma_start(out=out_r[co], in_=outt[co * 16:(co + 1) * 16, :])

    split_multiwaits(nc)
    return nc


def make_in_maps(x, weight, gamma, beta):
    x = np.ascontiguousarray(x, dtype=np.float32)
    weight = np.ascontiguousarray(weight, dtype=np.float32)
    gamma = np.ascontiguousarray(gamma, dtype=np.float32)
    beta = np.ascontiguousarray(beta, dtype=np.float32)
    selcor = np.zeros((128, 128), np.float32)
    for c in range(CO):
        selcor[c * 16:(c + 1) * 16, c * 16:(c + 1) * 16] = 1.0
    maps = []
    for c in range(8):
        sl = slice(c * CO, (c + 1) * CO)
        maps.append({
            "x": x,
            "w": np.ascontiguousarray(weight[sl]),
            "gamma": np.ascontiguousarray(gamma[sl]),
            "beta": np.ascontiguousarray(beta[sl]),
            "selcor": selcor,
        })
    return maps


def assemble(results):
    return np.concatenate([r["out"] for r in results], axis=1)


# ---------------------------------------------------------------------------
# Harness entry point: full inputs in, full output out.
# Sharding: output channels co split 8 ways (8 channels per NeuronCore);
# BN statistics are over the full batch, which each core owns for its
# channels, so no collectives are needed.
# ---------------------------------------------------------------------------
from concourse.bass_utils import run_bass_kernel_spmd

_NC_CACHE = None


def _get_nc():
    global _NC_CACHE
    if _NC_CACHE is None:
        _NC_CACHE = build_nc()
    return _NC_CACHE


def kernel(x, weight, gamma, beta):
    nc = _get_nc()
    in_maps = make_in_maps(np.asarray(x), np.asarray(weight),
                           np.asarray(gamma), np.asarray(beta))
    res = run_bass_kernel_spmd(nc, in_maps, core_ids=list(range(8)))
    return assemble(res.results)


# revision 26
# speedup vs baseline: 1.0857x; 1.0023x over previous
"""AdderNet BasicBlock kernel for Trainium2, co-sharded across 8 cores.

Per core (co-shard CO=8 of 64 output channels):
  conv[co,n,p] = -sum_{ci,kh,kw} |x[n,ci,p+k-1] - w[co,ci,kh,kw]|   (pad=1)
  BN train-mode over (n,h,w) per co, then ReLU.

Formulation (v2):
  Taps are split between engines:
    ACT taps (2 of 9): |d| = Abs(x + (-w)) directly  -> PE weight -1
    DVE taps (7 of 9): relu(x-w) = tensor_scalar(sub, max 0), bf16 4x mode
        |d| = 2*relu(d) - d            -> PE weight -2, Box correction +1
  conv = -(sum_ACT |d| + 2*sum_DVE relu(d)) + Box_D - SwD
  SwD is a per-channel constant -> dropped (BN is shift-invariant per channel).

PE: 4-way column tiling. Position j holds co j (tile A) / co 4+j (tile B);
ones-reduce over 128 partitions (2 image groups x 64 ci), 512-col blocks,
PSUM accumulates 9 taps. Box (7 DVE taps over x) streams into tile A after
its evacuation, at a per-half rotating position.

BN stage 2 as before: bounce conv to DRAM, reload as [(co,n), hw],
replicated-selector matmul stats, fused affine+relu on ACT.
"""
from contextlib import ExitStack

import numpy as np

import concourse.bass as bass
import concourse.tile as tile
import concourse.mybir as mybir

F32 = mybir.dt.float32
BF16 = mybir.dt.bfloat16
BN_EPS = 1e-5

N, CI, H, W = 16, 64, 32, 32
CO = 8          # output channels per core
HW = H * W      # 1024
PADH, PADW = H + 2, W + 4   # 34 x 36 (2 extra zero cols: alignment + even dims)

N_HALVES = 4
JPH = 2                      # images per group per half
TCOLS = JPH * HW             # 2048 free-dim per stream
NB = TCOLS // 512            # 512-col matmul blocks per stream

ACT_TAPS = (1, 7)            # (0,1),(2,1): kw=1 taps go to ACT (Abs direct)
# interleave ACT taps for engine overlap; DVE taps use xb0 (kw even) or xb1
TAP_ORDER = (0, 2, 1, 3, 4, 7, 5, 6, 8)


def split_multiwaits(nc, max_waits=1):
    """This container's walrus rejects >1 semaphore wait per instruction.
    Hoist extras into standalone NoOps on the same (in-order) engine."""
    n_split = 0
    for f in nc.m.functions:
        for b in f.blocks:
            insts = list(b.instructions)
            changed = False
            new = []
            for inst in insts:
                si = inst.sync_info
                waits = list(si.on_wait) if si and si.on_wait else []
                if len(waits) > max_waits:
                    changed = True
                    n_split += 1
                    for w in waits[: len(waits) - max_waits]:
                        new.append(mybir.InstNoOp(
                            name=nc.get_next_instruction_name(),
                            engine=inst.engine, ins=[], outs=[],
                            sync_info=mybir.SyncInfo(on_wait=[w], on_update=[]),
                        ))
                    inst.sync_info = mybir.SyncInfo(
                        on_wait=waits[len(waits) - max_waits:],
                        on_update=list(si.on_update) if si.on_update else [],
                    )
                new.append(inst)
            if changed:
                b.instructions = new
    return n_split


def build_nc(t_bufs=22, debug=False):
    """One core's SPMD program."""
    nc = bass.Bass()
    x = nc.declare_dram_parameter("x", [N, CI, H, W], F32, isOutput=False)
    w = nc.declare_dram_parameter("w", [CO, CI, 3, 3], F32, isOutput=False)
    gamma = nc.declare_dram_parameter("gamma", [CO], F32, isOutput=False)
    beta = nc.declare_dram_parameter("beta", [CO], F32, isOutput=False)
    selcor_in = nc.declare_dram_parameter("selcor", [128, 128], F32,
                                          isOutput=False)
    out = nc.declare_dram_parameter("out", [N, CO, H, W], F32, isOutput=True)
    if debug:
        dbg_cs = nc.declare_dram_parameter("dbg_cs", [128, HW], F32,
                                           isOutput=True)
        dbg_box = nc.declare_dram_parameter("dbg_box", [128, HW], F32,
                                            isOutput=True)

    with tile.TileContext(nc) as tc, ExitStack() as ctx:
        singles = ctx.enter_context(tc.tile_pool(name="singles", bufs=1))
        xspool = ctx.enter_context(tc.tile_pool(name="xstage", bufs=3))
        xbpool = ctx.enter_context(tc.tile_pool(name="xb", bufs=3))
        tpool = ctx.enter_context(tc.tile_pool(name="tpool", bufs=t_bufs))
        scpool = ctx.enter_context(tc.tile_pool(name="scr", bufs=2))
        sbpool = ctx.enter_context(tc.tile_pool(name="scrbox", bufs=2))
        pspool = ctx.enter_context(tc.tile_pool(name="psum", bufs=2,
                                                space="PSUM"))
        spool = ctx.enter_context(tc.tile_pool(name="stage2", bufs=1))
        dpool = ctx.enter_context(tc.tile_pool(name="dram", bufs=1,
                                               space="DRAM"))

        # ---- constants ----
        def ones_pair(val, nm):
            t = singles.tile([128, 2], BF16, name=nm)
            nc.vector.memset(t[:, :], 0.0)
            nc.vector.memset(t[0:64, 0:1], val)
            nc.vector.memset(t[64:128, 1:2], val)
            return t
        sel_m2 = ones_pair(-2.0, "sel_m2")   # DVE relu streams
        sel_m1 = ones_pair(-1.0, "sel_m1")   # ACT |d| streams
        sel_p1 = ones_pair(1.0, "sel_p1")    # box streams

        eps_t = singles.tile([128, 1], F32)
        nc.vector.memset(eps_t[:, :], BN_EPS)

        w_sb = singles.tile([128, CO * 9], F32)     # w_sb[(g,ci), co*9+tap]
        w_src = w.rearrange("co ci kh kw -> ci co (kh kw)")
        nc.sync.dma_start(
            out=w_sb[0:64, :].rearrange("p (co t) -> p co t", t=9), in_=w_src)
        nc.sync.dma_start(
            out=w_sb[64:128, :].rearrange("p (co t) -> p co t", t=9), in_=w_src)
        neg_w_sb = singles.tile([128, CO * 9], F32)
        nc.vector.tensor_scalar(
            out=neg_w_sb[:, :], in0=w_sb[:, :], scalar1=-1.0, scalar2=None,
            op0=mybir.AluOpType.mult)

        # stage-2 targets: row = co*16 + g*8 + h*2 + j (n-major per co)
        cs_rl = spool.tile([128, HW], F32)      # S
        box_rl = spool.tile([128, HW], F32)     # Box broadcast per co
        conv_d = dpool.tile([CO, 2, N_HALVES, JPH, HW], F32)
        box_d = dpool.tile([2, N_HALVES, JPH, HW], F32)

        def emit_xload(h):
            x_st = xspool.tile([128, JPH, PADH, PADW], F32, tag="xst",
                               name=f"xst{h}")
            if h < 3:   # borders stay zero across 3-deep buffer reuse
                nc.vector.memset(x_st[:, :, 0, :], 0.0)
                nc.vector.memset(x_st[:, :, PADH - 1, :], 0.0)
                nc.vector.memset(x_st[:, :, 1:PADH - 1, 0:1], 0.0)
                nc.vector.memset(x_st[:, :, 1:PADH - 1, H + 1:PADW], 0.0)
            for g in range(2):
                for jj in range(JPH):
                    nc.sync.dma_start(
                        out=x_st[g * 64:(g + 1) * 64, jj, 1:H + 1, 1:W + 1],
                        in_=x[g * 8 + h * JPH + jj])
            return x_st

        def emit_cast0(h, x_st):
            xb0 = xbpool.tile([128, JPH, PADH, PADW], BF16, tag="xb0",
                              name=f"xb0_{h}")
            nc.vector.tensor_copy(
                out=xb0.rearrange("p a h w -> p (a h w)"),
                in_=x_st.rearrange("p a h w -> p (a h w)"))
            return xb0

        def emit_cast1(h, x_st):
            xb1 = xbpool.tile([128, JPH, PADH, PADW], BF16, tag="xb1",
                              name=f"xb1_{h}")
            nc.scalar.copy(
                out=xb1[:, :, :, 0:PADW - 2],
                in_=x_st[:, :, :, 1:PADW - 1])
            return xb1

        # ---- stage 1 ----
        pendingB = None         # deferred slot-B evacuation from prior half
        xs0 = emit_xload(0)
        xbs = {0: [emit_cast0(0, xs0), emit_cast1(0, xs0)]}
        for half in range(N_HALVES):
            xb0, xb1 = xbs.pop(half)

            def tap_src(tap):
                kh, kw = divmod(tap, 3)
                if kw == 1 and tap not in ACT_TAPS:
                    return xb1, kh, 0        # shifted copy, aligned
                return xb0, kh, kw

            psA = pspool.tile([128, TCOLS], F32, tag="ps", name=f"psA{half}")
            psB = pspool.tile([128, TCOLS], F32, tag="ps", name=f"psB{half}")

            def emit_evac(ps, co0, h, slot):
                # evacuate, bounce via DRAM, reload into the stage-2 rows
                scr = scpool.tile([128, TCOLS], F32, tag="scr",
                                  name=f"scr{h}_{slot}")
                nc.scalar.copy(scr[:, :], ps[:, :])
                for cl in range(4):
                    co = co0 + cl
                    nc.sync.dma_start(
                        out=conv_d[co, :, h, :, :],
                        in_=scr[32 * cl:32 * cl + 2, :].rearrange(
                            "p (a hw) -> p a hw", hw=HW))
                    if h == N_HALVES - 1:
                        nc.sync.dma_start(
                            out=cs_rl[co * 16:(co + 1) * 16, :],
                            in_=conv_d[co].rearrange("g h j w -> (g h j) w"))

            def emit_box_mms(ps, h):
                # box streams: block b at position b (4-way concurrent)
                box_taps = [t for t in range(9) if t not in ACT_TAPS]
                for bi, tap in enumerate(box_taps):
                    src_t, skh, skw = tap_src(tap)
                    for b in range(NB):
                        a, hb = divmod(b, 2)
                        rhs = src_t[:, a, skh + hb * 16:skh + hb * 16 + 16,
                                    skw:skw + W]
                        nc.tensor.matmul(
                            ps[32 * b:32 * b + 2, b * 512:(b + 1) * 512],
                            lhsT=sel_p1[:, :], rhs=rhs,
                            start=(bi == 0), stop=(bi == len(box_taps) - 1),
                            tile_position=(0, 32 * b))

            def emit_box_evac(ps, h):
                scb = sbpool.tile([2, TCOLS], F32, tag="scb",
                                  name=f"scb{h}")
                for b in range(NB):
                    nc.scalar.copy(scb[:, b * 512:(b + 1) * 512],
                                   ps[32 * b:32 * b + 2, b * 512:(b + 1) * 512])
                nc.sync.dma_start(
                    out=box_d[:, h, :, :],
                    in_=scb.rearrange("p (a hw) -> p a hw", hw=HW))
                if h == N_HALVES - 1:
                    for c2 in range(CO):
                        nc.sync.dma_start(
                            out=box_rl[c2 * 16:(c2 + 1) * 16, :],
                            in_=box_d.rearrange("g h j w -> (g h j) w"))

            def emit_taps(ps, co0, deferred):
                """deferred: {ti: fn} emitted after tap-group ti's MMs."""
                for ti, tap in enumerate(TAP_ORDER):
                    src_t, skh, skw = tap_src(tap)
                    on_act = tap in ACT_TAPS
                    sel = sel_m1 if on_act else sel_m2
                    ts = []
                    for cl in range(4):
                        co = co0 + cl
                        k = co * 9 + tap
                        t = tpool.tile([128, JPH, H, W], BF16, tag="t",
                                       name=f"t{half}_{co0}_{ti}_{cl}")
                        src = src_t[:, :, skh:skh + H, skw:skw + W]
                        if on_act:
                            nc.scalar.activation(
                                out=t[:, :, :, :], in_=src,
                                func=mybir.ActivationFunctionType.Abs,
                                bias=neg_w_sb[:, k:k + 1], scale=1.0)
                        else:
                            nc.vector.tensor_scalar(
                                out=t[:, :, :, :], in0=src,
                                scalar1=w_sb[:, k:k + 1], scalar2=0.0,
                                op0=mybir.AluOpType.subtract,
                                op1=mybir.AluOpType.max)
                        ts.append(t.rearrange("p a h w -> p (a h w)"))
                    for b in range(NB):
                        for cl in range(4):
                            nc.tensor.matmul(
                                ps[32 * cl:32 * cl + 2,
                                   b * 512:(b + 1) * 512],
                                lhsT=sel[:, :],
                                rhs=ts[cl][:, b * 512:(b + 1) * 512],
                                start=(ti == 0), stop=(ti == 8),
                                tile_position=(0, 32 * cl))
                    fn = deferred.get(ti)
                    if fn is not None:
                        fn()

            # prefetch next half's x (DMA queue) + defer its casts into tapsA
            defA = {}
            if pendingB is not None:
                defA[0] = pendingB
            if half + 1 < N_HALVES:
                xs_n = emit_xload(half + 1)
                xbs[half + 1] = [None, None]

                def mk0(h1, xst):
                    def fn():
                        xbs[h1][0] = emit_cast0(h1, xst)
                    return fn

                def mk1(h1, xst):
                    def fn():
                        xbs[h1][1] = emit_cast1(h1, xst)
                    return fn
                defA[2] = mk0(half + 1, xs_n)
                defA[3] = mk1(half + 1, xs_n)
            emit_taps(psA, 0, defA)
            # tapsB: evacA after group 1, box MMs after group 2, box evac
            # after group 6 (box MMs done on PE by then)
            defB = {
                1: lambda: emit_evac(psA, 0, half, 0),
                2: lambda: emit_box_mms(psA, half),
                6: lambda: emit_box_evac(psA, half),
            }
            emit_taps(psB, 4, defB)
            if half == N_HALVES - 1:
                emit_evac(psB, 4, half, 1)
            else:
                pendingB = (lambda ps=psB, h=half:
                            emit_evac(ps, 4, h, 1))

        # ---- stage 2: BN stats + affine + relu ----
        selcor = singles.tile([128, 128], F32)      # replicated stats selector
        nc.sync.dma_start(out=selcor[:, :], in_=selcor_in[:, :])
        selcor_r = singles.tile([128, 128], BF16)
        nc.vector.tensor_copy(out=selcor_r[:, :], in_=selcor[:, :])
        gam = singles.tile([128, 1], F32)
        bet = singles.tile([128, 1], F32)
        for co in range(CO):
            nc.sync.dma_start(out=gam[co * 16:(co + 1) * 16, :],
                              in_=gamma[co:co + 1].partition_broadcast(16))
            nc.sync.dma_start(out=bet[co * 16:(co + 1) * 16, :],
                              in_=beta[co:co + 1].partition_broadcast(16))
        conv_rl = spool.tile([128, HW], F32)    # true conv output (+SwD shift)
        nc.vector.tensor_add(conv_rl[:, :], cs_rl[:, :], box_rl[:, :])
        if debug:
            nc.sync.dma_start(out=dbg_cs[:, :], in_=cs_rl[:, :])
            nc.sync.dma_start(out=dbg_box[:, :], in_=box_rl[:, :])

        # stats: mean via replicated-selector matmul, then centered var
        conv_r = spool.tile([128, HW], BF16)
        nc.vector.tensor_copy(out=conv_r[:, :], in_=conv_rl[:, :])
        ps1 = pspool.tile([128, 512], F32, tag="ps")
        ps1b = pspool.tile([128, 512], F32, tag="ps")
        nc.tensor.matmul(ps1[:, :], lhsT=selcor_r[:, :], rhs=conv_r[:, 0:512],
                         start=True, stop=True)
        nc.tensor.matmul(ps1b[:, :], lhsT=selcor_r[:, :], rhs=conv_r[:, 512:HW],
                         start=True, stop=True)
        s1 = spool.tile([128, 1], F32)
        s1b = spool.tile([128, 1], F32)
        nc.vector.tensor_reduce(out=s1[:, :], in_=ps1[:, :],
                                axis=mybir.AxisListType.X, op=mybir.AluOpType.add)
        nc.vector.tensor_reduce(out=s1b[:, :], in_=ps1b[:, :],
                                axis=mybir.AxisListType.X, op=mybir.AluOpType.add)
        inv_n = 1.0 / (N * HW)
        mean = spool.tile([128, 1], F32)
        nc.vector.tensor_scalar(out=mean[:, :], in0=s1[:, :], scalar1=s1b[:, :],
                                scalar2=inv_n, op0=mybir.AluOpType.add,
                                op1=mybir.AluOpType.mult)
        # centered square -> variance without cancellation
        dctr = spool.tile([128, HW], F32)
        nc.vector.tensor_scalar(out=dctr[:, :], in0=conv_rl[:, :],
                                scalar1=mean[:, :], scalar2=None,
                                op0=mybir.AluOpType.subtract)
        sq = spool.tile([128, HW], BF16)
        nc.scalar.activation(out=sq[:, :], in_=dctr[:, :],
                             func=mybir.ActivationFunctionType.Square)
        ps2 = pspool.tile([128, 512], F32, tag="ps")
        ps2b = pspool.tile([128, 512], F32, tag="ps")
        nc.tensor.matmul(ps2[:, :], lhsT=selcor_r[:, :], rhs=sq[:, 0:512],
                         start=True, stop=True)
        nc.tensor.matmul(ps2b[:, :], lhsT=selcor_r[:, :], rhs=sq[:, 512:HW],
                         start=True, stop=True)
        s2 = spool.tile([128, 1], F32)
        s2b = spool.tile([128, 1], F32)
        nc.vector.tensor_reduce(out=s2[:, :], in_=ps2[:, :],
                                axis=mybir.AxisListType.X, op=mybir.AluOpType.add)
        nc.vector.tensor_reduce(out=s2b[:, :], in_=ps2b[:, :],
                                axis=mybir.AxisListType.X, op=mybir.AluOpType.add)
        var = spool.tile([128, 1], F32)
        nc.vector.tensor_scalar(out=var[:, :], in0=s2[:, :], scalar1=s2b[:, :],
                                scalar2=inv_n, op0=mybir.AluOpType.add,
                                op1=mybir.AluOpType.mult)
        std = spool.tile([128, 1], F32)
        nc.scalar.activation(out=std[:, :], in_=var[:, :],
                             func=mybir.ActivationFunctionType.Sqrt,
                             bias=eps_t[:, :], scale=1.0)
        rstd = spool.tile([128, 1], F32)
        nc.vector.reciprocal(out=rstd[:, :], in_=std[:, :])
        a_t = spool.tile([128, 1], F32)
        nc.vector.tensor_mul(a_t[:, :], gam[:, :], rstd[:, :])
        ma = spool.tile([128, 1], F32)
        nc.vector.tensor_mul(ma[:, :], mean[:, :], a_t[:, :])
        b_t = spool.tile([128, 1], F32)
        nc.vector.tensor_sub(b_t[:, :], bet[:, :], ma[:, :])

        outt = spool.tile([128, HW], F32)
        nc.scalar.activation(out=outt[:, :], in_=conv_rl[:, :],
                             func=mybir.ActivationFunctionType.Relu,
                             bias=b_t[:, :], scale=a_t[:, :])
        out_r = out.rearrange("n co h w -> co n (h w)")
        for co in range(CO):
            nc.sync.dma_start(out=out_r[co], in_=outt[co * 16:(co + 1) * 16, :])

    split_multiwaits(nc)
    return nc


def make_in_maps(x, weight, gamma, beta):
    x = np.ascontiguousarray(x, dtype=np.float32)
    weight = np.ascontiguousarray(weight, dtype=np.float32)
    gamma = np.ascontiguousarray(gamma, dtype=np.float32)
    beta = np.ascontiguousarray(beta, dtype=np.float32)
    selcor = np.zeros((128, 128), np.float32)
    for c in range(CO):
        selcor[c * 16:(c + 1) * 16, c * 16:(c + 1) * 16] = 1.0
    maps = []
    for c in range(8):
        sl = slice(c * CO, (c + 1) * CO)
        maps.append({
            "x": x,
            "w": np.ascontiguousarray(weight[sl]),
            "gamma": np.ascontiguousarray(gamma[sl]),
            "beta": np.ascontiguousarray(beta[sl]),
            "selcor": selcor,
        })
    return maps


def assemble(results):
    return np.concatenate([r["out"] for r in results], axis=1)


# ---------------------------------------------------------------------------
# Harness entry point: full inputs in, full output out.
# Sharding: output channels co split 8 ways (8 channels per NeuronCore);
# BN statistics are over the full batch, which each core owns for its
# channels, so no collectives are needed.
# ---------------------------------------------------------------------------
from concourse.bass_utils import run_bass_kernel_spmd

_NC_CACHE = None


def _get_nc():
    global _NC_CACHE
    if _NC_CACHE is None:
        _NC_CACHE = build_nc()
    return _NC_CACHE


def kernel(x, weight, gamma, beta):
    nc = _get_nc()
    in_maps = make_in_maps(np.asarray(x), np.asarray(weight),
                           np.asarray(gamma), np.asarray(beta))
    res = run_bass_kernel_spmd(nc, in_maps, core_ids=list(range(8)))
    return assemble(res.results)
